# revision 50
# baseline (speedup 1.0000x reference)
"""GRACE contrastive loss on 8 Trainium2 NeuronCores (Bass/Tile).

loss = mean over i of 0.5*(l1_i + l2_i), where (T=0.5, a/b = row-normalized
h1/h2):
  l1_i = log(sum_j exp(a_i.a_j/T) - e^2 + sum_j exp(a_i.b_j/T)) - a_i.b_i/T
  l2_i = log(sum_j exp(b_i.b_j/T) - e^2 + sum_j exp(b_i.a_j/T)) - a_i.b_i/T

Work split over 8 cores, exploiting symmetry of the two reflexive
similarity matrices (only the upper/lower triangle of a@a.T / b@b.T is
exponentiated; the mirrored half is recovered from column sums):

- Phase B (all cores): rows c*1024..(c+1)*1024 of exp(a@b.T): matmul +
  exp with fused row-sum accumulation (ScalarE accum_out), exp values
  staged to SBUF in fp8 for column sums.
- Phase U (all cores): 9 "units" of 1024x1024. In the concatenated
  column-block space [a blocks 0-7 | b blocks 8-15], core c computes
  blocks c..c+8: that is rows a_c x upper-triangle columns of a, plus
  rows b_c x lower-triangle columns of b — a contiguous block run, so a
  single partition-id-derived register offset makes the program SPMD-
  uniform. Each unit emits row sums (accum_out) and column sums (VectorE
  tree-add over row tiles + ones-vector matmul partition reduce on PE).
- cs_ab groups: column sums of exp(a@b.T) via ones-matmuls over the fp8
  staging, PSUM-accumulated across row tiles, interleaved into phase U.

The host does the O(N*D) pieces: normalization, diag(a@b.T), final
assembly of row/column sums into the two denominators, log, mean.
"""

import hashlib
import inspect
import os
import pickle
import types
from contextlib import ExitStack
from pathlib import Path

import ml_dtypes
import numpy as np

TEMPERATURE = 0.5
EPS = 1e-8
N, D = 8192, 128
NCORES = 8
BLK = N // NCORES          # 1024 rows per core / unit side
RT = BLK // 128            # 8 row tiles per block
NU = 9                     # units per core in phase U


def _install_neff_disk_cache():
    """Cache walrus NEFF compiles on disk so fresh-process runs are fast."""
    import concourse.bass2jax as bass2jax

    if getattr(bass2jax, "_grace_neff_cache", False):
        return
    inner = bass2jax.compile_bir_kernel
    cache_dir = Path(os.environ.get("XDG_CACHE_HOME", os.path.expanduser("~/.cache")))
    cache_dir = cache_dir / "bass_neff_cache"
    try:
        cache_dir.mkdir(parents=True, exist_ok=True)
    except OSError:
        return

    def cached(bir_json, tmpdir, neff_name="file.neff"):
        data = bir_json if isinstance(bir_json, bytes) else bir_json.encode()
        key = hashlib.sha256(data).hexdigest()
        path = cache_dir / f"{key}_{neff_name}"
        out_path = os.path.join(tmpdir, neff_name)
        if path.exists():
            with open(path, "rb") as f:
                neff = f.read()
            with open(out_path, "wb") as f:
                f.write(neff)
            return out_path
        res = inner(bir_json, tmpdir, neff_name)
        try:
            with open(res, "rb") as f:
                neff = f.read()
            tmp = path.with_suffix(".tmp%d" % os.getpid())
            with open(tmp, "wb") as f:
                f.write(neff)
            tmp.rename(path)
        except OSError:
            pass
        return res

    bass2jax.compile_bir_kernel = cached
    bass2jax._grace_neff_cache = True


_PROGRAM = None


def build_program():
    global _PROGRAM
    if _PROGRAM is not None:
        return _PROGRAM

    import concourse.bass as bass
    import concourse.tile as tile
    from concourse import bacc, mybir

    BF = mybir.dt.bfloat16
    F8 = mybir.dt.float8e4
    F32 = mybir.dt.float32
    Exp = mybir.ActivationFunctionType.Exp
    X = mybir.AxisListType.X

    nc = bacc.Bacc(
        "TRN2",
        target_bir_lowering=False,
        debug=False,
        enable_asserts=False,
        num_devices=NCORES,
    )
    at_d = nc.dram_tensor("at", [128, N], BF, kind="ExternalInput").ap()
    bt_d = nc.dram_tensor("bt", [128, N], BF, kind="ExternalInput").ap()
    # single packed output per core: [rs_ab(8) | rs9(72) | cs_ab bits(32) |
    # cs9 bits(36)] f32 columns — one sharded transfer instead of four.
    NOUT = RT + NU * RT + N // 256 + NU * BLK // 256
    out_d = nc.dram_tensor("out", [128, NOUT], F32, kind="ExternalOutput").ap()
    rs_ab_d = out_d[:, 0:RT]
    rs9_d = out_d[:, RT : RT + NU * RT]
    cs_ab_d = out_d[:, RT + NU * RT : RT + NU * RT + N // 256]
    cs9_d = out_d[:, RT + NU * RT + N // 256 : NOUT]

    with tile.TileContext(nc) as tc, ExitStack() as ctx:
        inp = ctx.enter_context(tc.tile_pool(name="inp", bufs=1))
        expp = ctx.enter_context(tc.tile_pool(name="expst", bufs=1))
        ustp = ctx.enter_context(tc.tile_pool(name="ust", bufs=2))
        lhsp = ctx.enter_context(tc.tile_pool(name="lhst", bufs=2))
        accp = ctx.enter_context(tc.tile_pool(name="acc", bufs=4))
        rsp = ctx.enter_context(tc.tile_pool(name="rs", bufs=1))
        csbp = ctx.enter_context(tc.tile_pool(name="csb", bufs=1))
        onep = ctx.enter_context(tc.tile_pool(name="ones", bufs=1))

        # ---- input DMAs (first-use order) ----
        pid0 = nc.partition_id()
        PIECE = N // 8
        # this core's a-row-block, sliced out of the full at by partition id
        abt_t = inp.tile([128, BLK], BF)
        nc.sync.dma_start(abt_t[:], at_d[:, bass.ds(pid0 * BLK, BLK)])
        bt_p = []
        for i in range(8):
            t = inp.tile([128, PIECE], BF, tag=f"bt{i}")
            nc.sync.dma_start(t[:], bt_d[:, i * PIECE : (i + 1) * PIECE])
            bt_p.append(t)
        # concatenated [at | bt] column-block space for phase U
        atbt = inp.tile([128, 2 * N], BF)
        nc.sync.dma_start(atbt[:, 0:N], at_d[:])
        nc.sync.dma_start(atbt[:, N : 2 * N], bt_d[:])

        ones8 = onep.tile([128, 1], F8, tag="ones8")
        nc.vector.memset(ones8[:], 1.0)
        ones16 = onep.tile([128, 1], BF, tag="ones16")
        nc.vector.memset(ones16[:], 1.0)

        # fp8 staging of exp(a_blk@b^T) for the cs_ab column sums
        expst = expp.tile([128, RT * N], F8)
        cs_sb = csbp.tile([1, N], BF, tag="cs_sb")
        cs9_sb = csbp.tile([1, NU * BLK], BF, tag="cs9_sb")
        rs9_t = rsp.tile([128, NU * RT], F32, tag="rs9")

        pid = pid0

        # ---- Phase B: between slab, full width, 2048-column ACT chunks ----
        with tc.tile_pool(name="mmB", bufs=2, space="PSUM") as mmB:
            rs_t = rsp.tile([128, RT], F32, tag="rs_ab")
            # rt 0 starts with two 1024-column chunks so the first exp fires
            # as soon as the first bt DMA piece lands
            first_chunks = [(0, 1024), (1024, 1024), (2048, 2048), (4096, 2048), (6144, 2048)]
            rest_chunks = [(0, 2048), (2048, 2048), (4096, 2048), (6144, 2048)]
            for rt in range(RT):
                lhsT = abt_t[:, rt * 128 : (rt + 1) * 128]
                chunks = first_chunks if rt == 0 else rest_chunks
                acc = accp.tile([128, len(first_chunks)], F32)
                for ci, (coff, sz) in enumerate(chunks):
                    mt = mmB.tile([128, 2048], F32)
                    for q in range(sz // 512):
                        off = coff + q * 512
                        nc.tensor.matmul(
                            mt[:, q * 512 : (q + 1) * 512],
                            lhsT=lhsT,
                            rhs=bt_p[off // PIECE][:, off % PIECE : off % PIECE + 512],
                            start=True,
                            stop=True,
                        )
                    nc.scalar.activation(
                        expst[:, rt * N + coff : rt * N + coff + sz],
                        mt[:, :sz],
                        Exp,
                        scale=2.0,
                        accum_out=acc[:, ci : ci + 1],
                    )
                nc.vector.reduce_sum(
                    rs_t[:, rt : rt + 1], acc[:, : len(chunks)], axis=X
                )
            nc.sync.dma_start(rs_ab_d[:], rs_t[:])

        # ---- Phase U: 9 symmetric units + interleaved cs_ab groups ----
        with (
            tc.tile_pool(name="mmU", bufs=2, space="PSUM") as mmU,
            tc.tile_pool(name="ucs", bufs=1, space="PSUM") as ucs,
            tc.tile_pool(name="csp", bufs=1, space="PSUM") as csp,
            tc.tile_pool(name="wcs", bufs=1, space="PSUM") as wcsp,
        ):

            def csab_group(ct):
                cst = csp.tile([1, 512], F32)
                for rt in range(RT):
                    nc.tensor.matmul(
                        cst[:, :],
                        lhsT=ones8[:, :],
                        rhs=expst[:, rt * N + ct * 512 : rt * N + (ct + 1) * 512],
                        start=(rt == 0),
                        stop=(rt == RT - 1),
                    )
                nc.vector.tensor_copy(cs_sb[:, ct * 512 : (ct + 1) * 512], cst[:, :])

            csab_sched = iter(range(N // 512))
            for u in range(NU):
                # unit's column block in [at|bt] space: t = pid + u.
                # rows: a-block pid when t < 8 (upper triangle of a@a.T),
                # b-block pid when t >= 8 (lower triangle of b@b.T).
                base = (pid + u) * BLK
                lhsoff = (pid + (((pid + u) & 8))) * BLK
                lhst = lhsp.tile([128, BLK], BF)
                nc.vector.tensor_copy(lhst[:, :], atbt[:, bass.ds(lhsoff, BLK)])
                ust = ustp.tile([128, RT * BLK], BF)
                is_diag = u in (0, NU - 1)  # aa / bb diagonal block for every core
                if is_diag:
                    # upper triangle only (row tile rt x cols >= rt*128); the
                    # mirrored lower part of each row sum is recovered from
                    # per-128-column-window column sums accumulated in PSUM.
                    pw = wcsp.tile([1, BLK], F32)
                    for rt in range(RT):
                        w = BLK - rt * 128
                        mt = mmU.tile([128, BLK], F32)
                        for q in range((w + 511) // 512):
                            qw = min(512, w - q * 512)
                            nc.tensor.matmul(
                                mt[:, q * 512 : q * 512 + qw],
                                lhsT=lhst[:, rt * 128 : (rt + 1) * 128],
                                rhs=atbt[:, bass.ds(base + rt * 128 + q * 512, qw)],
                                start=True,
                                stop=True,
                            )
                        nc.scalar.activation(
                            ust[:, rt * BLK : rt * BLK + w],
                            mt[:, :w],
                            Exp,
                            scale=2.0,
                            accum_out=rs9_t[:, u * RT + rt : u * RT + rt + 1],
                        )
                        for wq in range(rt + 1, RT):
                            lo = rt * BLK + (wq - rt) * 128
                            # start=True clears has_written for the WHOLE psum
                            # bank, so only the first matmul touching each of
                            # pw's two banks may set it; later first-writes hit
                            # cleared has_written bits and overwrite-then-
                            # accumulate naturally.
                            nc.tensor.matmul(
                                pw[:, wq * 128 : (wq + 1) * 128],
                                lhsT=ones16[:, :],
                                rhs=ust[:, lo : lo + 128],
                                start=(rt == 0 and wq in (1, 4)),
                                stop=(rt == wq - 1),
                                skip_group_check=True,
                            )
                    # windows 1..7 -> the (otherwise unused) diag cs9 slot
                    dst = cs9_sb[:, u * BLK + 128 : (u + 1) * BLK]
                    if u == NU - 1:
                        nc.scalar.copy(dst, pw[:, 128:BLK])
                    else:
                        nc.vector.tensor_copy(dst, pw[:, 128:BLK])
                    for _ in range(2):
                        ct = next(csab_sched, None)
                        if ct is not None:
                            csab_group(ct)
                    continue
                for rt in range(RT):
                    mt = mmU.tile([128, BLK], F32)
                    for q in range(2):
                        nc.tensor.matmul(
                            mt[:, q * 512 : (q + 1) * 512],
                            lhsT=lhst[:, rt * 128 : (rt + 1) * 128],
                            rhs=atbt[:, bass.ds(base + q * 512, 512)],
                            start=True,
                            stop=True,
                        )
                    nc.scalar.activation(
                        ust[:, rt * BLK : (rt + 1) * BLK],
                        mt[:, :],
                        Exp,
                        scale=2.0,
                        accum_out=rs9_t[:, u * RT + rt : u * RT + rt + 1],
                    )
                    # column sums, step 1: running prefix add on DVE — each
                    # add fires right after its row tile's exp, so the
                    # partition-reduce below has almost nothing left to wait on.
                    if rt > 0:
                        nc.vector.tensor_add(
                            ust[:, rt * BLK : (rt + 1) * BLK],
                            ust[:, rt * BLK : (rt + 1) * BLK],
                            ust[:, (rt - 1) * BLK : rt * BLK],
                        )
                for h in range(2):
                    uc = ucs.tile([1, 512], F32)
                    nc.tensor.matmul(
                        uc[:, :],
                        lhsT=ones16[:, :],
                        rhs=ust[:, 7 * BLK + h * 512 : 7 * BLK + (h + 1) * 512],
                        start=True,
                        stop=True,
                    )
                    # last unit: use the (by then idle) scalar engine so the
                    # kernel tail isn't serialized behind DVE copies
                    dst = cs9_sb[:, u * BLK + h * 512 : u * BLK + (h + 1) * 512]
                    if u == NU - 1:
                        nc.scalar.copy(dst, uc[:, :])
                    else:
                        nc.vector.tensor_copy(dst, uc[:, :])
                # interleave ~2 cs_ab groups per unit
                for _ in range(2):
                    ct = next(csab_sched, None)
                    if ct is not None:
                        csab_group(ct)
            for ct in csab_sched:
                csab_group(ct)

        nc.sync.dma_start(rs9_d[:], rs9_t[:])
        nc.sync.dma_start(cs9_d[:], cs9_sb[:].bitcast(F32))
        nc.sync.dma_start(cs_ab_d[:], cs_sb[:].bitcast(F32))

    nc.compile()
    _PROGRAM = nc
    return nc


def _cache_root():
    d = Path(os.environ.get("XDG_CACHE_HOME", os.path.expanduser("~/.cache")))
    return d / "bass_neff_cache"


_META = None


def _get_program_meta():
    """BIR bytes + IO metadata for the program; builds the Bass program only
    on (disk-)cache miss, so warm processes skip the ~1s bass/Tile build."""
    global _META
    if _META is not None:
        return _META
    src = inspect.getsource(build_program) + "|meta_v3"
    key = hashlib.sha256(src.encode()).hexdigest()[:24]
    path = _cache_root() / f"grace_prog_{key}.pkl"
    if path.exists():
        try:
            with open(path, "rb") as f:
                _META = pickle.load(f)
            return _META
        except Exception:
            pass
    nc = build_program()
    from concourse import mybir

    pname = nc.partition_id_tensor.name if nc.partition_id_tensor else None
    ins, outs = [], []
    for alloc in nc.m.functions[0].allocations:
        if not isinstance(alloc, mybir.MemoryLocationSet):
            continue
        name = alloc.memorylocations[0].name
        if alloc.kind == "ExternalInput":
            if name != pname:
                ins.append(name)
        elif alloc.kind == "ExternalOutput":
            # NOTE: keep the np.dtype object itself — .str is '<V2' for
            # ml_dtypes bfloat16 and does not round-trip.
            outs.append((name, tuple(alloc.tensor_shape), np.dtype(mybir.dt.np(alloc.dtype))))
    _META = {
        "bir": nc.to_json_bytes(),
        "arch": nc.m.arch,
        "ins": ins,
        "outs": outs,
        "pname": pname,
    }
    try:
        path.parent.mkdir(parents=True, exist_ok=True)
        tmp = path.with_suffix(".tmp%d" % os.getpid())
        with open(tmp, "wb") as f:
            pickle.dump(_META, f)
        tmp.rename(path)
    except OSError:
        pass
    return _META


class _NcShim:
    """Duck-typed stand-in for the Bass object in _bass_exec_p lowering."""

    def __init__(self, meta):
        self._bir = meta["bir"]
        self.m = types.SimpleNamespace(arch=meta["arch"])
        self.target_bir_lowering = False
        self.has_collectives = False
        self.dbg_addr = None
        self.dbg_callbacks = ()

    def to_json_bytes(self):
        return self._bir

    def is_finalized(self):
        return True


_JITTED = None


def _exe_cache_path(meta):
    import jax

    key = hashlib.sha256(
        meta["bir"] + jax.__version__.encode() + b"|exe_v1"
    ).hexdigest()[:24]
    return _cache_root() / f"grace_exe_{key}.pkl"


def _build_compiled(meta, sample_args):
    """Trace+compile the sharded executable, serialize it to disk."""
    import jax
    import concourse.bass2jax as b2j
    from jax.experimental.shard_map import shard_map
    from jax.sharding import Mesh, PartitionSpec

    out_names = [n for n, _, _ in meta["outs"]]
    b2j.install_neuronx_cc_hook()
    shim = _NcShim(meta)
    out_avals = tuple(
        jax.core.ShapedArray(s, np.dtype(d)) for _, s, d in meta["outs"]
    )
    in_names = tuple(meta["ins"]) + tuple(out_names)
    if meta["pname"]:
        in_names = in_names + (meta["pname"],)
    n_params = len(meta["ins"])
    n_outs = len(out_names)

    def _body(*args):
        operands = list(args)
        if meta["pname"]:
            operands.append(b2j.partition_id_tensor())
        outs = b2j._bass_exec_p.bind(
            *operands,
            out_avals=out_avals,
            in_names=in_names,
            out_names=tuple(out_names),
            lowering_input_output_aliases=(),
            sim_require_finite=True,
            sim_require_nnan=True,
            nc=shim,
        )
        return tuple(outs)

    devices = jax.devices()[:NCORES]
    mesh = Mesh(np.asarray(devices), ("core",))
    in_specs = (PartitionSpec(),) * n_params + (PartitionSpec("core"),) * n_outs
    out_specs = (PartitionSpec("core"),) * n_outs
    jitted = jax.jit(
        shard_map(
            _body, mesh=mesh, in_specs=in_specs, out_specs=out_specs, check_rep=False
        ),
        donate_argnums=tuple(range(n_params, n_params + n_outs)),
        keep_unused=True,
    )
    compiled = jitted.lower(*sample_args).compile()
    try:
        from jax.experimental import serialize_executable as se

        blob = se.serialize(compiled)
        path = _exe_cache_path(meta)
        path.parent.mkdir(parents=True, exist_ok=True)
        tmp = path.with_suffix(".tmp%d" % os.getpid())
        with open(tmp, "wb") as f:
            pickle.dump(blob, f)
        tmp.rename(path)
    except Exception:
        pass
    return compiled


def _zeros_for(meta):
    return [np.zeros((NCORES * s[0], *s[1:]), np.dtype(d)) for _, s, d in meta["outs"]]


def _run(meta, at, bt):
    """Run the program on 8 cores: at/bt replicated (uploaded once), outputs
    sharded per core. Returns {name: array[NCORES, *shape]}."""
    global _JITTED
    zeros = _zeros_for(meta)
    if _JITTED is None:
        exe_path = _exe_cache_path(meta)
        if exe_path.exists():
            try:
                from jax.experimental import serialize_executable as se

                with open(exe_path, "rb") as f:
                    blob = pickle.load(f)
                _JITTED = se.deserialize_and_load(*blob)
            except Exception:
                _JITTED = None
        if _JITTED is None:
            _JITTED = _build_compiled(meta, (at, bt, *zeros))
    outs = _JITTED(at, bt, *zeros)
    return {
        n: np.asarray(o).reshape(NCORES, *spec[1])
        for n, o, spec in zip([n for n, _, _ in meta["outs"]], outs, meta["outs"])
    }


def _normalize(x):
    n = np.linalg.norm(x, axis=1, keepdims=True)
    return x / np.maximum(n, EPS)


def _warmup():
    """Import-time priming: initialize the device backend, load the cached
    executable and run one dummy dispatch so the first real call is fast."""
    try:
        import jax

        jax.devices()
        _install_neff_disk_cache()
        meta = _get_program_meta()
        bf = ml_dtypes.bfloat16
        z = np.zeros((128, N), bf)
        _run(meta, z, z)
    except Exception:
        pass


# Synchronous: a background thread racing the caller's own jax work has been
# observed to wedge the device (NRT_EXEC_UNIT_UNRECOVERABLE).
if os.environ.get("GRACE_NO_WARMUP", "0") != "1":
    _warmup()


def kernel(h1: np.ndarray, h2: np.ndarray):
    h1 = np.asarray(h1, dtype=np.float32)
    h2 = np.asarray(h2, dtype=np.float32)
    assert h1.shape == (N, D) and h2.shape == (N, D)

    a = _normalize(h1)
    b = _normalize(h2)
    diag = np.einsum("ij,ij->i", a, b, dtype=np.float64)

    bf = ml_dtypes.bfloat16
    at = np.ascontiguousarray(a.T).astype(bf)   # [128, 8192]
    bt = np.ascontiguousarray(b.T).astype(bf)

    _install_neff_disk_cache()
    try:
        results = _run(_get_program_meta(), at, bt)
    except Exception as e:
        import traceback

        print(f"grace fast path failed ({e!r}); falling back", flush=True)
        traceback.print_exc()
        # Robust fallback: full build + stock SPMD runner.
        nc = build_program()
        from concourse import bass_utils

        in_maps = [{"at": at, "bt": bt} for _ in range(NCORES)]
        r = bass_utils.run_bass_kernel_spmd(nc, in_maps, core_ids=list(range(NCORES)))
        results = {"out": np.stack([r.results[c]["out"] for c in range(NCORES)])}

    # ---- host assembly ----
    # row-tile layout [128, RT] -> rows: global row = rt*128 + p
    def rows_of(arr):  # [128, k*RT] -> [k, BLK]
        k = arr.shape[1] // RT
        return arr.astype(np.float64).T.reshape(k, RT, 128).reshape(k, BLK)

    def unbits(arr):  # [128, k] f32 region -> flat bf16 [k*256] as f64
        flat = np.ascontiguousarray(arr).reshape(-1).view(ml_dtypes.bfloat16)
        return flat.astype(np.float64)

    packed = results["out"]  # [NCORES, 128, NOUT]
    c0, c1, c2 = RT, RT + NU * RT, RT + NU * RT + N // 256

    e2 = np.exp(2.0)
    rs_ab = np.concatenate(
        [rows_of(packed[c][:, 0:c0])[0] for c in range(NCORES)]
    )
    cs_ab = np.sum([unbits(packed[c][:, c1:c2]) for c in range(NCORES)], axis=0)

    rs_aa = np.zeros(N, dtype=np.float64)
    rs_bb = np.zeros(N, dtype=np.float64)
    for c in range(NCORES):
        rs9 = rows_of(packed[c][:, c0:c1])   # [NU, BLK] row sums per unit
        cs9 = unbits(packed[c][:, c2:])      # [NU*BLK] col sums per unit
        # diagonal units computed only the upper triangle; complete each row
        # with the mirrored window column sums stashed in the diag cs9 slot
        for u in (0, NU - 1):
            rs9[u][128:] += cs9[u * BLK + 128 : (u + 1) * BLK]
        for u in range(NU):
            t = c + u  # column block in [a 0-7 | b 8-15] space
            if t < NCORES:
                # unit of a@a.T: rows block c, columns block t (t >= c)
                rs_aa[c * BLK : (c + 1) * BLK] += rs9[u]
                if u > 0:  # mirrored half: contributes to rows block t
                    rs_aa[t * BLK : (t + 1) * BLK] += cs9[u * BLK : (u + 1) * BLK]
            else:
                # unit of b@b.T: rows block c, columns block v (v <= c)
                v = t - NCORES
                rs_bb[c * BLK : (c + 1) * BLK] += rs9[u]
                if v < c:  # mirrored half: contributes to rows block v
                    rs_bb[v * BLK : (v + 1) * BLK] += cs9[u * BLK : (u + 1) * BLK]

    denom1 = rs_aa - e2 + rs_ab
    denom2 = rs_bb - e2 + cs_ab
    l1 = np.log(denom1) - 2.0 * diag
    l2 = np.log(denom2) - 2.0 * diag
    loss = np.mean(0.5 * (l1 + l2))
    return (np.asarray(loss, dtype=np.float32), 1)


# revision 51
# speedup vs baseline: 1.5337x; 1.5337x over previous
"""GRACE contrastive loss on 8 Trainium2 NeuronCores (Bass/Tile).

loss = mean over i of 0.5*(l1_i + l2_i), where (T=0.5, a/b = row-normalized
h1/h2):
  l1_i = log(sum_j exp(a_i.a_j/T) - e^2 + sum_j exp(a_i.b_j/T)) - a_i.b_i/T
  l2_i = log(sum_j exp(b_i.b_j/T) - e^2 + sum_j exp(b_i.a_j/T)) - a_i.b_i/T

Work split over 8 cores, exploiting symmetry of the two reflexive
similarity matrices (only the upper/lower triangle of a@a.T / b@b.T is
exponentiated; the mirrored half is recovered from column sums):

- Phase B (all cores): rows c*1024..(c+1)*1024 of exp(a@b.T): matmul +
  exp with fused row-sum accumulation (ScalarE accum_out), exp values
  staged to SBUF in fp8 for column sums.
- Phase U (all cores): 9 "units" of 1024x1024. In the concatenated
  column-block space [a blocks 0-7 | b blocks 8-15], core c computes
  blocks c..c+8: that is rows a_c x upper-triangle columns of a, plus
  rows b_c x lower-triangle columns of b — a contiguous block run, so a
  single partition-id-derived register offset makes the program SPMD-
  uniform. Each unit emits row sums (accum_out) and column sums (VectorE
  tree-add over row tiles + ones-vector matmul partition reduce on PE).
- cs_ab groups: column sums of exp(a@b.T) via ones-matmuls over the fp8
  staging, PSUM-accumulated across row tiles, interleaved into phase U.

The host does the O(N*D) pieces: normalization, diag(a@b.T), final
assembly of row/column sums into the two denominators, log, mean.
"""

import hashlib
import inspect
import os
import pickle
import types
from contextlib import ExitStack
from pathlib import Path

import ml_dtypes
import numpy as np

TEMPERATURE = 0.5
EPS = 1e-8
N, D = 8192, 128
NCORES = 8
BLK = N // NCORES          # 1024 rows per core / unit side
RT = BLK // 128            # 8 row tiles per block
NU = 9                     # units per core in phase U


def _install_neff_disk_cache():
    """Cache walrus NEFF compiles on disk so fresh-process runs are fast."""
    import concourse.bass2jax as bass2jax

    if getattr(bass2jax, "_grace_neff_cache", False):
        return
    inner = bass2jax.compile_bir_kernel
    cache_dir = Path(os.environ.get("XDG_CACHE_HOME", os.path.expanduser("~/.cache")))
    cache_dir = cache_dir / "bass_neff_cache"
    try:
        cache_dir.mkdir(parents=True, exist_ok=True)
    except OSError:
        return

    def cached(bir_json, tmpdir, neff_name="file.neff"):
        data = bir_json if isinstance(bir_json, bytes) else bir_json.encode()
        key = hashlib.sha256(data).hexdigest()
        path = cache_dir / f"{key}_{neff_name}"
        out_path = os.path.join(tmpdir, neff_name)
        if path.exists():
            with open(path, "rb") as f:
                neff = f.read()
            with open(out_path, "wb") as f:
                f.write(neff)
            return out_path
        res = inner(bir_json, tmpdir, neff_name)
        try:
            with open(res, "rb") as f:
                neff = f.read()
            tmp = path.with_suffix(".tmp%d" % os.getpid())
            with open(tmp, "wb") as f:
                f.write(neff)
            tmp.rename(path)
        except OSError:
            pass
        return res

    bass2jax.compile_bir_kernel = cached
    bass2jax._grace_neff_cache = True


_PROGRAM = None


def build_program():
    global _PROGRAM
    if _PROGRAM is not None:
        return _PROGRAM

    import concourse.bass as bass
    import concourse.tile as tile
    from concourse import bacc, mybir

    BF = mybir.dt.bfloat16
    F8 = mybir.dt.float8e4
    F32 = mybir.dt.float32
    Exp = mybir.ActivationFunctionType.Exp
    X = mybir.AxisListType.X

    nc = bacc.Bacc(
        "TRN2",
        target_bir_lowering=False,
        debug=False,
        enable_asserts=False,
        num_devices=NCORES,
    )
    at_d = nc.dram_tensor("at", [128, N], F8, kind="ExternalInput").ap()
    bt_d = nc.dram_tensor("bt", [128, N], F8, kind="ExternalInput").ap()
    # single packed output per core: [rs_ab(8) | rs9(72) | cs_ab bits(32) |
    # cs9 bits(36)] f32 columns — one sharded transfer instead of four.
    NOUT = RT + NU * RT + N // 256 + NU * BLK // 256
    out_d = nc.dram_tensor("out", [128, NOUT], F32, kind="ExternalOutput").ap()
    rs_ab_d = out_d[:, 0:RT]
    rs9_d = out_d[:, RT : RT + NU * RT]
    cs_ab_d = out_d[:, RT + NU * RT : RT + NU * RT + N // 256]
    cs9_d = out_d[:, RT + NU * RT + N // 256 : NOUT]

    with tile.TileContext(nc) as tc, ExitStack() as ctx:
        inp = ctx.enter_context(tc.tile_pool(name="inp", bufs=1))
        expp = ctx.enter_context(tc.tile_pool(name="expst", bufs=1))
        ustp = ctx.enter_context(tc.tile_pool(name="ust", bufs=2))
        lhsp = ctx.enter_context(tc.tile_pool(name="lhst", bufs=2))
        accp = ctx.enter_context(tc.tile_pool(name="acc", bufs=4))
        rsp = ctx.enter_context(tc.tile_pool(name="rs", bufs=1))
        csbp = ctx.enter_context(tc.tile_pool(name="csb", bufs=1))
        onep = ctx.enter_context(tc.tile_pool(name="ones", bufs=1))

        # ---- input DMAs (first-use order) ----
        pid0 = nc.partition_id()
        PIECE = N // 8
        # this core's a-row-block, sliced out of the full at by partition id
        abt_t = inp.tile([128, BLK], F8)
        nc.sync.dma_start(abt_t[:], at_d[:, bass.ds(pid0 * BLK, BLK)])
        bt_p = []
        for i in range(8):
            t = inp.tile([128, PIECE], F8, tag=f"bt{i}")
            nc.sync.dma_start(t[:], bt_d[:, i * PIECE : (i + 1) * PIECE])
            bt_p.append(t)
        # concatenated [at | bt] column-block space for phase U
        atbt = inp.tile([128, 2 * N], F8)
        nc.sync.dma_start(atbt[:, 0:N], at_d[:])
        nc.sync.dma_start(atbt[:, N : 2 * N], bt_d[:])

        ones8 = onep.tile([128, 1], F8, tag="ones8")
        nc.vector.memset(ones8[:], 1.0)
        ones16 = onep.tile([128, 1], BF, tag="ones16")
        nc.vector.memset(ones16[:], 1.0)

        # fp8 staging of exp(a_blk@b^T) for the cs_ab column sums
        expst = expp.tile([128, RT * N], F8)
        cs_sb = csbp.tile([1, N], BF, tag="cs_sb")
        cs9_sb = csbp.tile([1, NU * BLK], BF, tag="cs9_sb")
        rs9_t = rsp.tile([128, NU * RT], F32, tag="rs9")

        pid = pid0

        # ---- Phase B: between slab, full width, 2048-column ACT chunks ----
        with tc.tile_pool(name="mmB", bufs=2, space="PSUM") as mmB:
            rs_t = rsp.tile([128, RT], F32, tag="rs_ab")
            # rt 0 starts with two 1024-column chunks so the first exp fires
            # as soon as the first bt DMA piece lands
            first_chunks = [(0, 1024), (1024, 1024), (2048, 2048), (4096, 2048), (6144, 2048)]
            rest_chunks = [(0, 2048), (2048, 2048), (4096, 2048), (6144, 2048)]
            for rt in range(RT):
                lhsT = abt_t[:, rt * 128 : (rt + 1) * 128]
                chunks = first_chunks if rt == 0 else rest_chunks
                acc = accp.tile([128, len(first_chunks)], F32)
                for ci, (coff, sz) in enumerate(chunks):
                    mt = mmB.tile([128, 2048], F32)
                    for q in range(sz // 512):
                        off = coff + q * 512
                        nc.tensor.matmul(
                            mt[:, q * 512 : (q + 1) * 512],
                            lhsT=lhsT,
                            rhs=bt_p[off // PIECE][:, off % PIECE : off % PIECE + 512],
                            start=True,
                            stop=True,
                        )
                    nc.scalar.activation(
                        expst[:, rt * N + coff : rt * N + coff + sz],
                        mt[:, :sz],
                        Exp,
                        scale=2.0,
                        accum_out=acc[:, ci : ci + 1],
                    )
                nc.vector.reduce_sum(
                    rs_t[:, rt : rt + 1], acc[:, : len(chunks)], axis=X
                )
            nc.sync.dma_start(rs_ab_d[:], rs_t[:])

        # ---- Phase U: 9 symmetric units + interleaved cs_ab groups ----
        with (
            tc.tile_pool(name="mmU", bufs=2, space="PSUM") as mmU,
            tc.tile_pool(name="ucs", bufs=1, space="PSUM") as ucs,
            tc.tile_pool(name="csp", bufs=1, space="PSUM") as csp,
            tc.tile_pool(name="wcs", bufs=1, space="PSUM") as wcsp,
        ):

            def csab_group(ct):
                cst = csp.tile([1, 512], F32)
                for rt in range(RT):
                    nc.tensor.matmul(
                        cst[:, :],
                        lhsT=ones8[:, :],
                        rhs=expst[:, rt * N + ct * 512 : rt * N + (ct + 1) * 512],
                        start=(rt == 0),
                        stop=(rt == RT - 1),
                    )
                nc.vector.tensor_copy(cs_sb[:, ct * 512 : (ct + 1) * 512], cst[:, :])

            csab_sched = iter(range(N // 512))
            for u in range(NU):
                # unit's column block in [at|bt] space: t = pid + u.
                # rows: a-block pid when t < 8 (upper triangle of a@a.T),
                # b-block pid when t >= 8 (lower triangle of b@b.T).
                base = (pid + u) * BLK
                lhsoff = (pid + (((pid + u) & 8))) * BLK
                lhst = lhsp.tile([128, BLK], F8)
                nc.vector.tensor_copy(lhst[:, :], atbt[:, bass.ds(lhsoff, BLK)])
                ust = ustp.tile([128, RT * BLK], BF)
                is_diag = u in (0, NU - 1)  # aa / bb diagonal block for every core
                if is_diag:
                    # upper triangle only (row tile rt x cols >= rt*128); the
                    # mirrored lower part of each row sum is recovered from
                    # per-128-column-window column sums accumulated in PSUM.
                    pw = wcsp.tile([1, BLK], F32)
                    for rt in range(RT):
                        w = BLK - rt * 128
                        mt = mmU.tile([128, BLK], F32)
                        for q in range((w + 511) // 512):
                            qw = min(512, w - q * 512)
                            nc.tensor.matmul(
                                mt[:, q * 512 : q * 512 + qw],
                                lhsT=lhst[:, rt * 128 : (rt + 1) * 128],
                                rhs=atbt[:, bass.ds(base + rt * 128 + q * 512, qw)],
                                start=True,
                                stop=True,
                            )
                        nc.scalar.activation(
                            ust[:, rt * BLK : rt * BLK + w],
                            mt[:, :w],
                            Exp,
                            scale=2.0,
                            accum_out=rs9_t[:, u * RT + rt : u * RT + rt + 1],
                        )
                        for wq in range(rt + 1, RT):
                            lo = rt * BLK + (wq - rt) * 128
                            # start=True clears has_written for the WHOLE psum
                            # bank, so only the first matmul touching each of
                            # pw's two banks may set it; later first-writes hit
                            # cleared has_written bits and overwrite-then-
                            # accumulate naturally.
                            nc.tensor.matmul(
                                pw[:, wq * 128 : (wq + 1) * 128],
                                lhsT=ones16[:, :],
                                rhs=ust[:, lo : lo + 128],
                                start=(rt == 0 and wq in (1, 4)),
                                stop=(rt == wq - 1),
                                skip_group_check=True,
                            )
                    # windows 1..7 -> the (otherwise unused) diag cs9 slot
                    dst = cs9_sb[:, u * BLK + 128 : (u + 1) * BLK]
                    if u == NU - 1:
                        nc.scalar.copy(dst, pw[:, 128:BLK])
                    else:
                        nc.vector.tensor_copy(dst, pw[:, 128:BLK])
                    for _ in range(2):
                        ct = next(csab_sched, None)
                        if ct is not None:
                            csab_group(ct)
                    continue
                for rt in range(RT):
                    mt = mmU.tile([128, BLK], F32)
                    for q in range(2):
                        nc.tensor.matmul(
                            mt[:, q * 512 : (q + 1) * 512],
                            lhsT=lhst[:, rt * 128 : (rt + 1) * 128],
                            rhs=atbt[:, bass.ds(base + q * 512, 512)],
                            start=True,
                            stop=True,
                        )
                    nc.scalar.activation(
                        ust[:, rt * BLK : (rt + 1) * BLK],
                        mt[:, :],
                        Exp,
                        scale=2.0,
                        accum_out=rs9_t[:, u * RT + rt : u * RT + rt + 1],
                    )
                    # column sums, step 1: running prefix add on DVE — each
                    # add fires right after its row tile's exp, so the
                    # partition-reduce below has almost nothing left to wait on.
                    if rt > 0:
                        nc.vector.tensor_add(
                            ust[:, rt * BLK : (rt + 1) * BLK],
                            ust[:, rt * BLK : (rt + 1) * BLK],
                            ust[:, (rt - 1) * BLK : rt * BLK],
                        )
                for h in range(2):
                    uc = ucs.tile([1, 512], F32)
                    nc.tensor.matmul(
                        uc[:, :],
                        lhsT=ones16[:, :],
                        rhs=ust[:, 7 * BLK + h * 512 : 7 * BLK + (h + 1) * 512],
                        start=True,
                        stop=True,
                    )
                    # last unit: use the (by then idle) scalar engine so the
                    # kernel tail isn't serialized behind DVE copies
                    dst = cs9_sb[:, u * BLK + h * 512 : u * BLK + (h + 1) * 512]
                    if u == NU - 1:
                        nc.scalar.copy(dst, uc[:, :])
                    else:
                        nc.vector.tensor_copy(dst, uc[:, :])
                # interleave ~2 cs_ab groups per unit
                for _ in range(2):
                    ct = next(csab_sched, None)
                    if ct is not None:
                        csab_group(ct)
            for ct in csab_sched:
                csab_group(ct)

        nc.sync.dma_start(rs9_d[:], rs9_t[:])
        nc.sync.dma_start(cs9_d[:], cs9_sb[:].bitcast(F32))
        nc.sync.dma_start(cs_ab_d[:], cs_sb[:].bitcast(F32))

    nc.compile()
    _PROGRAM = nc
    return nc


def _cache_root():
    d = Path(os.environ.get("XDG_CACHE_HOME", os.path.expanduser("~/.cache")))
    return d / "bass_neff_cache"


_META = None


def _get_program_meta():
    """BIR bytes + IO metadata for the program; builds the Bass program only
    on (disk-)cache miss, so warm processes skip the ~1s bass/Tile build."""
    global _META
    if _META is not None:
        return _META
    src = inspect.getsource(build_program) + "|meta_v3"
    key = hashlib.sha256(src.encode()).hexdigest()[:24]
    path = _cache_root() / f"grace_prog_{key}.pkl"
    if path.exists():
        try:
            with open(path, "rb") as f:
                _META = pickle.load(f)
            return _META
        except Exception:
            pass
    nc = build_program()
    from concourse import mybir

    pname = nc.partition_id_tensor.name if nc.partition_id_tensor else None
    ins, outs = [], []
    for alloc in nc.m.functions[0].allocations:
        if not isinstance(alloc, mybir.MemoryLocationSet):
            continue
        name = alloc.memorylocations[0].name
        if alloc.kind == "ExternalInput":
            if name != pname:
                ins.append(name)
        elif alloc.kind == "ExternalOutput":
            # NOTE: keep the np.dtype object itself — .str is '<V2' for
            # ml_dtypes bfloat16 and does not round-trip.
            outs.append((name, tuple(alloc.tensor_shape), np.dtype(mybir.dt.np(alloc.dtype))))
    _META = {
        "bir": nc.to_json_bytes(),
        "arch": nc.m.arch,
        "ins": ins,
        "outs": outs,
        "pname": pname,
    }
    try:
        path.parent.mkdir(parents=True, exist_ok=True)
        tmp = path.with_suffix(".tmp%d" % os.getpid())
        with open(tmp, "wb") as f:
            pickle.dump(_META, f)
        tmp.rename(path)
    except OSError:
        pass
    return _META


class _NcShim:
    """Duck-typed stand-in for the Bass object in _bass_exec_p lowering."""

    def __init__(self, meta):
        self._bir = meta["bir"]
        self.m = types.SimpleNamespace(arch=meta["arch"])
        self.target_bir_lowering = False
        self.has_collectives = False
        self.dbg_addr = None
        self.dbg_callbacks = ()

    def to_json_bytes(self):
        return self._bir

    def is_finalized(self):
        return True


_JITTED = None


def _exe_cache_path(meta):
    import jax

    key = hashlib.sha256(
        meta["bir"] + jax.__version__.encode() + b"|exe_v1"
    ).hexdigest()[:24]
    return _cache_root() / f"grace_exe_{key}.pkl"


def _build_compiled(meta, sample_args):
    """Trace+compile the sharded executable, serialize it to disk."""
    import jax
    import concourse.bass2jax as b2j
    from jax.experimental.shard_map import shard_map
    from jax.sharding import Mesh, PartitionSpec

    out_names = [n for n, _, _ in meta["outs"]]
    b2j.install_neuronx_cc_hook()
    shim = _NcShim(meta)
    out_avals = tuple(
        jax.core.ShapedArray(s, np.dtype(d)) for _, s, d in meta["outs"]
    )
    in_names = tuple(meta["ins"]) + tuple(out_names)
    if meta["pname"]:
        in_names = in_names + (meta["pname"],)
    n_params = len(meta["ins"])
    n_outs = len(out_names)

    def _body(*args):
        operands = list(args)
        if meta["pname"]:
            operands.append(b2j.partition_id_tensor())
        outs = b2j._bass_exec_p.bind(
            *operands,
            out_avals=out_avals,
            in_names=in_names,
            out_names=tuple(out_names),
            lowering_input_output_aliases=(),
            sim_require_finite=True,
            sim_require_nnan=True,
            nc=shim,
        )
        return tuple(outs)

    devices = jax.devices()[:NCORES]
    mesh = Mesh(np.asarray(devices), ("core",))
    in_specs = (PartitionSpec(),) * n_params + (PartitionSpec("core"),) * n_outs
    out_specs = (PartitionSpec("core"),) * n_outs
    jitted = jax.jit(
        shard_map(
            _body, mesh=mesh, in_specs=in_specs, out_specs=out_specs, check_rep=False
        ),
        donate_argnums=tuple(range(n_params, n_params + n_outs)),
        keep_unused=True,
    )
    compiled = jitted.lower(*sample_args).compile()
    try:
        from jax.experimental import serialize_executable as se

        blob = se.serialize(compiled)
        path = _exe_cache_path(meta)
        path.parent.mkdir(parents=True, exist_ok=True)
        tmp = path.with_suffix(".tmp%d" % os.getpid())
        with open(tmp, "wb") as f:
            pickle.dump(blob, f)
        tmp.rename(path)
    except Exception:
        pass
    return compiled


def _zeros_for(meta):
    return [np.zeros((NCORES * s[0], *s[1:]), np.dtype(d)) for _, s, d in meta["outs"]]


def _run(meta, at, bt):
    """Run the program on 8 cores: at/bt replicated (uploaded once), outputs
    sharded per core. Returns {name: array[NCORES, *shape]}."""
    global _JITTED
    zeros = _zeros_for(meta)
    if _JITTED is None:
        exe_path = _exe_cache_path(meta)
        if exe_path.exists():
            try:
                from jax.experimental import serialize_executable as se

                with open(exe_path, "rb") as f:
                    blob = pickle.load(f)
                _JITTED = se.deserialize_and_load(*blob)
            except Exception:
                _JITTED = None
        if _JITTED is None:
            _JITTED = _build_compiled(meta, (at, bt, *zeros))
    outs = _JITTED(at, bt, *zeros)
    return {
        n: np.asarray(o).reshape(NCORES, *spec[1])
        for n, o, spec in zip([n for n, _, _ in meta["outs"]], outs, meta["outs"])
    }


def _normalize(x):
    n = np.linalg.norm(x, axis=1, keepdims=True)
    return x / np.maximum(n, EPS)


def _warmup():
    """Import-time priming: initialize the device backend, load the cached
    executable and run one dummy dispatch so the first real call is fast."""
    try:
        import jax

        jax.devices()
        _install_neff_disk_cache()
        meta = _get_program_meta()
        z = np.zeros((128, N), ml_dtypes.float8_e4m3)
        _run(meta, z, z)
    except Exception:
        pass


# Synchronous: a background thread racing the caller's own jax work has been
# observed to wedge the device (NRT_EXEC_UNIT_UNRECOVERABLE).
if os.environ.get("GRACE_NO_WARMUP", "0") != "1":
    _warmup()


def kernel(h1: np.ndarray, h2: np.ndarray):
    h1 = np.asarray(h1, dtype=np.float32)
    h2 = np.asarray(h2, dtype=np.float32)
    assert h1.shape == (N, D) and h2.shape == (N, D)

    a = _normalize(h1)
    b = _normalize(h2)
    diag = np.einsum("ij,ij->i", a, b, dtype=np.float64)

    f8 = ml_dtypes.float8_e4m3
    at = np.ascontiguousarray(a.T).astype(f8)   # [128, 8192]
    bt = np.ascontiguousarray(b.T).astype(f8)

    _install_neff_disk_cache()
    try:
        results = _run(_get_program_meta(), at, bt)
    except Exception as e:
        import traceback

        print(f"grace fast path failed ({e!r}); falling back", flush=True)
        traceback.print_exc()
        # Robust fallback: full build + stock SPMD runner.
        nc = build_program()
        from concourse import bass_utils

        in_maps = [{"at": at, "bt": bt} for _ in range(NCORES)]
        r = bass_utils.run_bass_kernel_spmd(nc, in_maps, core_ids=list(range(NCORES)))
        results = {"out": np.stack([r.results[c]["out"] for c in range(NCORES)])}

    # ---- host assembly ----
    # row-tile layout [128, RT] -> rows: global row = rt*128 + p
    def rows_of(arr):  # [128, k*RT] -> [k, BLK]
        k = arr.shape[1] // RT
        return arr.astype(np.float64).T.reshape(k, RT, 128).reshape(k, BLK)

    def unbits(arr):  # [128, k] f32 region -> flat bf16 [k*256] as f64
        flat = np.ascontiguousarray(arr).reshape(-1).view(ml_dtypes.bfloat16)
        return flat.astype(np.float64)

    packed = results["out"]  # [NCORES, 128, NOUT]
    c0, c1, c2 = RT, RT + NU * RT, RT + NU * RT + N // 256

    e2 = np.exp(2.0)
    rs_ab = np.concatenate(
        [rows_of(packed[c][:, 0:c0])[0] for c in range(NCORES)]
    )
    cs_ab = np.sum([unbits(packed[c][:, c1:c2]) for c in range(NCORES)], axis=0)

    rs_aa = np.zeros(N, dtype=np.float64)
    rs_bb = np.zeros(N, dtype=np.float64)
    for c in range(NCORES):
        rs9 = rows_of(packed[c][:, c0:c1])   # [NU, BLK] row sums per unit
        cs9 = unbits(packed[c][:, c2:])      # [NU*BLK] col sums per unit
        # diagonal units computed only the upper triangle; complete each row
        # with the mirrored window column sums stashed in the diag cs9 slot
        for u in (0, NU - 1):
            rs9[u][128:] += cs9[u * BLK + 128 : (u + 1) * BLK]
        for u in range(NU):
            t = c + u  # column block in [a 0-7 | b 8-15] space
            if t < NCORES:
                # unit of a@a.T: rows block c, columns block t (t >= c)
                rs_aa[c * BLK : (c + 1) * BLK] += rs9[u]
                if u > 0:  # mirrored half: contributes to rows block t
                    rs_aa[t * BLK : (t + 1) * BLK] += cs9[u * BLK : (u + 1) * BLK]
            else:
                # unit of b@b.T: rows block c, columns block v (v <= c)
                v = t - NCORES
                rs_bb[c * BLK : (c + 1) * BLK] += rs9[u]
                if v < c:  # mirrored half: contributes to rows block v
                    rs_bb[v * BLK : (v + 1) * BLK] += cs9[u * BLK : (u + 1) * BLK]

    denom1 = rs_aa - e2 + rs_ab
    denom2 = rs_bb - e2 + cs_ab
    l1 = np.log(denom1) - 2.0 * diag
    l2 = np.log(denom2) - 2.0 * diag
    loss = np.mean(0.5 * (l1 + l2))
    return (np.asarray(loss, dtype=np.float32), 1)
